# revision 68
# baseline (speedup 1.0000x reference)
"""Trainium2 Bass kernel for LoRA multi-head attention (B=2, S=2048, D=768, H=12, R=8).

Sharding over 8 cores: (batch, query-half, head-half) -> each core computes
6 heads x 1024 query rows x full 2048 keys, producing a partial (over the
head dimension) of the final merge projection. Host sums the two head-half
partials per (batch, query-half) slice.

All activations are kept feature-major ("transposed") on device so every
matmul contraction lands on the partition axis with no on-device transposes.
Host pre-packs every DRAM tensor so each DMA is one contiguous run per
partition (single descriptor per partition), split across the sync, scalar
and gpsimd queues.

The emission is software-pipelined around a continuous scalar-engine exp
stream (the steady-state bottleneck: 12.6M score elements per core):
half-pair scores (8 key tiles x 2 concurrently row-tiled heads) feed exp
directly into att tiles; mask multiplies are deferred, batched DVE ops;
v-projection and merge chunks ride a filler queue inside the scores halves
so the PE never idles while the scores PSUM ring is Act-paced. attv
matmuls append a shared all-ones block to v so each accumulation also
produces the softmax denominator replicated across 64 PSUM rows; the
normalize is then copy+reciprocal+one multiply on DVE. Dummy LDWEIGHTS
padding keeps the PE HAM clock-gate at full rate through Act-bound
stretches.
"""

import sys

if "/opt/trn_rl_repo" not in sys.path:
    sys.path.insert(0, "/opt/trn_rl_repo")

import numpy as np

import concourse.bass as bass
import concourse.tile as tile
from concourse import bacc, mybir
from concourse.bass_utils import run_bass_kernel_spmd

F32 = mybir.dt.float32
F32R = mybir.dt.float32r
BF16 = mybir.dt.bfloat16
I32 = mybir.dt.int32
EXP = mybir.ActivationFunctionType.Exp

B, S, D, H, R = 2, 2048, 768, 12, 8
DK = D // H  # 64
NCORES = 8
HPC = 6            # heads per core
HDIM = HPC * DK    # 384: head-slice width per core
QR = S // 2        # 1024 query rows per core
SC = 512           # streaming chunk (s dimension)
NSC = S // SC      # 4
NQC = QR // SC     # 2 query chunks per core
NKT = S // 128     # 16 key tiles
DO = D // 128      # 6 d-chunks

_CACHE = {}


def _build_kernel():
    """Build the full Bass program. One SPMD program serves all 8 cores; the
    (batch, q-half, head-half) selection is done host-side via input slicing.

    Emission order is software-pipelined so the scalar engine's exp chain
    starts right after the k/q projections, overlapping the v projection:
      pass1: k-proj, BW, uv, q-proj
      scores(q0,p0), scores(q1,p0)
      pass2: uq + v-proj
      attv/scores interleaved tail, merges
    """
    nc = bacc.Bacc("TRN2", target_bir_lowering=False, debug=False,
                   enable_asserts=True, num_devices=NCORES)

    def din(name, shape, dt=BF16):
        return nc.dram_tensor(name, shape, dt, kind="ExternalInput").ap()

    # all big inputs pre-packed host-side: leading chunk dim, then partition
    kt4_d = din("kt4", [NSC, 128, DO, SC])
    qt4_d = din("qt4", [NSC, 128, DO, SC])
    vth2_d = din("vth2", [NQC, 128, DO, SC])
    mask2_d = din("mask2", [NQC, 128, NKT, SC])
    wqt_d = din("wqt", [128, DO, HDIM])
    wkt_d = din("wkt", [128, DO, HDIM])
    wvt_d = din("wvt", [128, DO, HDIM])
    wmt_d = din("wmt", [128, 3, D])
    bqc_d = din("bqc", [128, 3], F32)
    bkc_d = din("bkc", [128, 3], F32)
    bvr_d = din("bvr", [1, HDIM])
    bmr_d = din("bmr", [1, D])
    aq_d = din("aq", [128, DO, R])
    av_d = din("av", [128, DO, R])
    bqt_d = din("bqt", [128, DO, R])
    bvt_d = din("bvt", [128, DO, R])
    out_d = nc.dram_tensor("out", [QR, D], F32, kind="ExternalOutput").ap()

    with tile.TileContext(nc) as tc:
        with (
            tc.tile_pool(name="keep", bufs=1) as keep,
            tc.tile_pool(name="maskp", bufs=2) as maskp,
            tc.tile_pool(name="wa", bufs=1) as wa,
            tc.tile_pool(name="acts", bufs=2) as acts,
            tc.tile_pool(name="kpool", bufs=2) as kpool,
            tc.tile_pool(name="bpool", bufs=2) as bpool,
            tc.tile_pool(name="apool", bufs=5) as apool,
            tc.tile_pool(name="npool", bufs=1) as npool,
            tc.tile_pool(name="fpool", bufs=2) as fpool,
            tc.tile_pool(name="psproj", bufs=2, space="PSUM") as psproj,
            tc.tile_pool(name="pss", bufs=2, space="PSUM") as pss,
            tc.tile_pool(name="pso", bufs=2, space="PSUM") as pso,
        ):
            # ---- persistent tiles ----
            ones_sb = keep.tile([1, 128], BF16)
            nc.vector.memset(ones_sb[:], 1.0)
            bm_sb = keep.tile([1, D], BF16)
            # kT split per s-chunk so scores can start on partial projections
            kT_sb = [keep.tile([128, 3, SC], BF16, name=f"kT{i}")
                     for i in range(NSC)]
            qT_sb = [keep.tile([128, 3, SC], BF16, name=f"qT{i}")
                     for i in range(NQC)]
            # v_aug per head pair: [v_even | ones | v_odd]; head 2p reads
            # cols 0:128 (num rows 0-63, den rows 64-127), head 2p+1 reads
            # cols 64:192 (den rows 0-63, num rows 64-127). The shared ones
            # block makes every attv matmul M=128 and yields the softmax
            # denominator replicated across 64 PSUM partitions for free.
            v_aug = keep.tile([128, NKT, HPC // 2, 3 * DK], BF16)
            nc.vector.memset(v_aug[:, :, :, DK:2 * DK], 1.0)

            # ---- prologue DMAs ----
            # sync queue: the k-projection critical path first; kTc2/3 go on
            # the scalar queue so their buffer-reuse waits don't block the
            # vThc loads behind them (head-of-line blocking).
            kTcs = []
            t = kpool.tile([128, DO, SC], BF16, tag="act", name="kTc0")
            nc.sync.dma_start(t[:], kt4_d[0])
            kTcs.append(t)
            wk_sb = wa.tile([128, DO, HDIM], BF16, name="wk_sb")
            nc.sync.dma_start(wk_sb[:], wkt_d[:])
            bk_sb = wa.tile([128, 3], F32)
            nc.sync.dma_start(bk_sb[:], bkc_d[:])
            bq_sb = wa.tile([128, 3], F32)
            nc.sync.dma_start(bq_sb[:], bqc_d[:])
            t = kpool.tile([128, DO, SC], BF16, tag="act", name="kTc1")
            nc.sync.dma_start(t[:], kt4_d[1])
            kTcs.append(t)
            vThcs = []
            for qc in range(NQC):
                t = acts.tile([128, DO, SC], BF16, tag="act2", name=f"vThc{qc}")
                nc.sync.dma_start(t[:], vth2_d[qc])
                vThcs.append(t)
            # kTc2/3 reuse ring buffers; scalar queue so their waits don't
            # block the sync queue
            for sc in (2, 3):
                t = kpool.tile([128, DO, SC], BF16, tag="act", name=f"kTc{sc}")
                nc.scalar.dma_start(t[:], kt4_d[sc])
                kTcs.append(t)

            # gpsimd queue: weights, lora, masks (otherwise idle engine)
            def gload(name, dram, shape, dt=BF16):
                t = wa.tile(shape, dt, name=name)
                nc.gpsimd.dma_start(t[:], dram[:])
                return t

            # ordered by when each transfer is first needed on-device
            wq_sb = gload("wq_sb", wqt_d, [128, DO, HDIM])
            bqt_sb = gload("bqt_sb", bqt_d, [128, DO, R])
            bvt_sb = gload("bvt_sb", bvt_d, [128, DO, R])
            av_sb = gload("av_sb", av_d, [128, DO, R])
            m01s = {}
            m01 = maskp.tile([128, NKT, SC], BF16, tag="mb", name="m01")
            nc.gpsimd.dma_start(m01[:], mask2_d[0])
            m01s[0] = m01
            bv_sb = gload("bv_sb", bvr_d, [1, HDIM])
            wv_sb = gload("wv_sb", wvt_d, [128, DO, HDIM])
            m01 = maskp.tile([128, NKT, SC], BF16, tag="mb", name="m01")
            nc.gpsimd.dma_start(m01[:], mask2_d[1])
            m01s[1] = m01
            aq_sb = gload("aq_sb", aq_d, [128, DO, R])
            nc.gpsimd.dma_start(bm_sb[:], bmr_d[:])

            BWq_sb = wa.tile([R, HDIM], BF16)
            BWv_sb = wa.tile([R, HDIM], BF16)

            # ---- pass 1 helpers ----
            def emit_kproj(sc):
                kTc = kTcs[sc]
                for et in range(3):
                    esl = slice(et * 128, (et + 1) * 128)
                    ps = psproj.tile([128, 512], F32, tag="pp", name="ps_k")
                    for do in range(DO):
                        nc.tensor.matmul(ps[:], wk_sb[:, do, esl], kTc[:, do, :],
                                         start=(do == 0), stop=(do == DO - 1))
                    nc.vector.tensor_scalar_add(
                        kT_sb[sc][:, et, :], ps[:], bk_sb[:, et:et + 1])

            def emit_bwq():
                ps = psproj.tile([128, 512], F32, tag="pp", name="ps_bwq")
                for do in range(DO):
                    nc.tensor.matmul(ps[:R, :HDIM], bvt_sb[:, do, :],
                                     wq_sb[:, do, :],
                                     start=(do == 0), stop=(do == DO - 1))
                nc.vector.tensor_copy(BWq_sb[:], ps[:R, :HDIM])

            def emit_bwv():
                ps = psproj.tile([128, 512], F32, tag="pp", name="ps_bwv")
                for do in range(DO):
                    nc.tensor.matmul(ps[:R, :HDIM], bqt_sb[:, do, :],
                                     wv_sb[:, do, :],
                                     start=(do == 0), stop=(do == DO - 1))
                nc.vector.tensor_copy(BWv_sb[:], ps[:R, :HDIM])

            def emit_qproj(qc):
                qsl = slice(qc * SC, (qc + 1) * SC)
                vThc = vThcs[qc]
                ps = psproj.tile([128, 512], F32, tag="pp", name="ps_uv")
                for do in range(DO):
                    nc.tensor.matmul(ps[:R, :], av_sb[:, do, :], vThc[:, do, :],
                                     start=(do == 0), stop=(do == DO - 1))
                uv_t = npool.tile([R, SC], BF16, tag="uv", name="uv_t")
                nc.vector.tensor_copy(uv_t[:], ps[:R, :])
                for et in range(3):
                    esl = slice(et * 128, (et + 1) * 128)
                    ps = psproj.tile([128, 512], F32, tag="pp", name="ps_q")
                    for do in range(DO):
                        nc.tensor.matmul(ps[:], wq_sb[:, do, esl], vThc[:, do, :],
                                         start=(do == 0), stop=False)
                    nc.tensor.matmul(ps[:], BWq_sb[:, esl], uv_t[:],
                                     start=False, stop=True)
                    nc.vector.tensor_scalar_add(
                        qT_sb[qc][:, et, :], ps[:], bq_sb[:, et:et + 1])

            # ---- phase B helpers ----
            atts = {}

            pso_tiles = {}
            HKT = NKT // 2  # 8 key tiles per half

            fillers = []

            def fill(n):
                for _ in range(min(n, len(fillers))):
                    fillers.pop(0)()

            def flush_fillers():
                while fillers:
                    fillers.pop(0)()

            def emit_scores_half(qc, p, hh, nfill=0, pad=False,
                                 mask_inline=False):
                # heads 2p (rows 0-63) and 2p+1 (rows 64-127) interleave as PE
                # row-groups and run concurrently; exp writes straight into
                # the att tile — the mask multiply is applied in-place later
                # (emit_mask_half) so it doesn't couple the DVE FIFO to the
                # exp stream. Filler units (pass2/merge chunks) are emitted
                # between kt groups so the PE stays busy while the scores
                # PSUM ring is paced by the Act engine.
                att = apool.tile([128, HKT, 2, SC], BF16, tag="att", name="att")
                atts[(qc, p, hh)] = att
                for i, kt in enumerate(range(hh * HKT, (hh + 1) * HKT)):
                    ps_s = pss.tile([128, 2, SC], F32, tag="ss", name="ps_s")
                    for j in range(2):
                        po = j * 64
                        nc.tensor.matmul(
                            ps_s[:, j, :],
                            kT_sb[kt // 4][po:po + 64, p,
                                           (kt % 4) * 128:(kt % 4 + 1) * 128],
                            qT_sb[qc][po:po + 64, p, :],
                            start=True, stop=True)
                    nc.scalar.activation(att[:, kt - hh * HKT, :, :], ps_s[:],
                                         EXP, scale=0.125)
                    if mask_inline:
                        ktl = kt - hh * HKT
                        mk = m01s[qc][:, kt, None, :].to_broadcast([128, 2, SC])
                        nc.vector.tensor_mul(att[:, ktl, :, :],
                                             att[:, ktl, :, :], mk)
                    if i < nfill:
                        fill(1)
                    elif pad:
                        for _ in range(4):
                            nc.tensor.ldweights(ones_sb[:])

            def emit_mask_half(qc, p, hh):
                m01 = m01s[qc]
                att = atts[(qc, p, hh)]
                for half in range(2):
                    a = half * (HKT // 2)
                    b = a + HKT // 2
                    mk = m01[:, hh * HKT + a:hh * HKT + b, None, :] \
                        .to_broadcast([128, HKT // 2, 2, SC])
                    nc.vector.tensor_mul(att[:, a:b, :, :], att[:, a:b, :, :],
                                         mk)

            def emit_attv_half(qc, p, hh):
                # attv accumulation spans both halves of the same PSUM tile;
                # other matmuls to other banks may interleave in between.
                att = atts.pop((qc, p, hh))
                for j in range(2):
                    if hh == 0:
                        pso_tiles[(qc, p, j)] = pso.tile([128, SC], F32,
                                                         tag="oo", name="ps_o")
                    ps_o = pso_tiles[(qc, p, j)]
                    for kt in range(hh * HKT, (hh + 1) * HKT):
                        nc.tensor.matmul(
                            ps_o[:],
                            v_aug[:, kt, p, j * DK:j * DK + 128],
                            att[:, kt - hh * HKT, j, :],
                            start=(kt == 0), stop=(kt == NKT - 1))
                    if hh == 1:
                        h = 2 * p + j
                        po = j * 64
                        # j=0: num rows 0-63, den rows 64-127; j=1 swapped.
                        no, do_ = (0, 64) if j == 0 else (64, 0)
                        ps_o = pso_tiles.pop((qc, p, j))
                        den_s = npool.tile([DK, SC], F32, tag="dn", name="den_s")
                        nc.vector.tensor_copy(den_s[:], ps_o[do_:do_ + DK, :])
                        rb = npool.tile([DK, SC], F32, tag="rb", name="rb")
                        nc.vector.reciprocal_approx_fast(rb[:], den_s[:])
                        for _ in range(4):
                            nc.tensor.ldweights(ones_sb[:])
                        if h == 0:
                            _OUTT[qc] = bpool.tile([128, 3, SC], BF16,
                                                   tag="outT", name="outT")
                        outT_sb = _OUTT[qc]
                        nc.vector.tensor_mul(outT_sb[po:po + 64, p, :],
                                             ps_o[no:no + DK, :], rb[:])

            def merge_chunk(qc, qt, ec):
                outT_sb = _OUTT[qc]
                qtsl = slice(qt * 128, (qt + 1) * 128)
                esl = slice(ec * 384, (ec + 1) * 384)
                ps_m = psproj.tile([128, 512], F32, tag="pp", name="ps_m")
                for hp in range(3):
                    nc.tensor.matmul(ps_m[:, :384], outT_sb[:, hp, qtsl],
                                     wm_sb[:, hp, esl],
                                     start=(hp == 0), stop=False)
                nc.tensor.matmul(ps_m[:, :384], ones_sb[:], bm_sb[:, esl],
                                 start=False, stop=True)
                fin = fpool.tile([128, 384], F32, tag="fin", name="fin")
                nc.vector.tensor_copy(fin[:], ps_m[:, :384])
                nc.sync.dma_start(
                    out_d[qc * SC + qt * 128:qc * SC + (qt + 1) * 128, esl],
                    fin[:])
                for _ in range(3):
                    nc.tensor.ldweights(ones_sb[:])

            def emit_merge(qc):
                for qt in range(4):
                    for ec in range(2):
                        merge_chunk(qc, qt, ec)

            def push_merge(qc):
                for qt in range(4):
                    for ec in range(2):
                        fillers.append(
                            lambda qt=qt, ec=ec: merge_chunk(qc, qt, ec))

            # ---- pass 2: uq + v-projection (natural layout), interleaved
            # with the first scores pairs so ACT/DVE fill early without the
            # projection copybacks queueing behind the mask multiplies ----
            qTcs = {}

            def emit_pass2_dma(sc):
                qTc = acts.tile([128, DO, SC], BF16, tag="act2", name="qTc")
                nc.sync.dma_start(qTc[:], qt4_d[sc])
                qTcs[sc] = qTc

            uqts = {}

            def p2_uq(sc):
                qTc = qTcs[sc]
                ps = psproj.tile([128, 512], F32, tag="pp", name="ps_uq")
                for do in range(DO):
                    nc.tensor.matmul(ps[:R, :], aq_sb[:, do, :], qTc[:, do, :],
                                     start=(do == 0), stop=(do == DO - 1))
                uq_t = npool.tile([R, SC], BF16, tag="uq", name="uq_t")
                nc.vector.tensor_copy(uq_t[:], ps[:R, :])
                uqts[sc] = uq_t

            def p2_st(sc, st):
                qTc = qTcs[sc]
                uq_t = uqts[sc]
                gst = sc * 4 + st
                stsl = slice(st * 128, (st + 1) * 128)
                ps = psproj.tile([128, 512], F32, tag="pp", name="ps_v")
                for do in range(DO):
                    nc.tensor.matmul(ps[:, :HDIM], qTc[:, do, stsl],
                                     wv_sb[:, do, :],
                                     start=(do == 0), stop=False)
                nc.tensor.matmul(ps[:, :HDIM],
                                 uq_t[:, st * 128:(st + 1) * 128], BWv_sb[:],
                                 start=False, stop=False)
                nc.tensor.matmul(ps[:, :HDIM], ones_sb[:], bv_sb[:],
                                 start=False, stop=True)
                sv = ps[:, :HDIM].rearrange("p (g two d) -> p g two d",
                                            g=3, two=2)
                nc.vector.tensor_copy(v_aug[:, gst, :, 0:DK], sv[:, :, 0, :])
                nc.vector.tensor_copy(v_aug[:, gst, :, 2 * DK:3 * DK],
                                      sv[:, :, 1, :])

            def push_p2(sc):
                fillers.append(lambda: p2_uq(sc))
                for st in range(4):
                    fillers.append(lambda st=st: p2_st(sc, st))

            # ---- schedule: projections first (DVE copybacks run while the
            # Act engine is still idle), then a continuous exp stream with
            # attv/pass2/merge work packed under it ----
            emit_kproj(0)
            emit_kproj(1)
            emit_bwq()
            emit_qproj(0)
            emit_scores_half(0, 0, 0)
            emit_qproj(1)
            emit_pass2_dma(0)
            emit_pass2_dma(1)
            emit_scores_half(1, 0, 0)
            emit_kproj(2)
            emit_kproj(3)
            wm_sb = wk_sb[:].rearrange("p (u v) c -> p u (v c)", u=3, v=2)
            nc.gpsimd.dma_start(wm_sb, wmt_d[:])
            emit_scores_half(0, 0, 1)
            emit_bwv()
            push_p2(0)
            push_p2(1)
            emit_scores_half(1, 0, 1, nfill=8)
            fill(2)
            emit_pass2_dma(2)
            push_p2(2)
            emit_mask_half(0, 0, 0)
            emit_attv_half(0, 0, 0)
            emit_scores_half(0, 1, 0, nfill=8)
            emit_pass2_dma(3)
            push_p2(3)
            emit_scores_half(0, 1, 1, nfill=8)
            flush_fillers()
            emit_mask_half(0, 0, 1)
            emit_attv_half(0, 0, 1)
            emit_mask_half(1, 0, 0)
            emit_attv_half(1, 0, 0)
            emit_scores_half(1, 1, 0, pad=True)
            emit_mask_half(1, 0, 1)
            emit_attv_half(1, 0, 1)
            emit_scores_half(1, 1, 1, pad=True)
            emit_mask_half(0, 1, 0)
            emit_attv_half(0, 1, 0)
            emit_scores_half(0, 2, 0, pad=True, mask_inline=True)
            emit_mask_half(0, 1, 1)
            emit_attv_half(0, 1, 1)
            emit_scores_half(0, 2, 1, pad=True, mask_inline=True)
            emit_mask_half(1, 1, 0)
            emit_attv_half(1, 1, 0)
            emit_scores_half(1, 2, 0, pad=True, mask_inline=True)
            emit_mask_half(1, 1, 1)
            emit_attv_half(1, 1, 1)
            emit_scores_half(1, 2, 1, pad=True, mask_inline=True)
            emit_attv_half(0, 2, 0)
            emit_attv_half(0, 2, 1)
            emit_merge(0)
            emit_attv_half(1, 2, 0)
            emit_attv_half(1, 2, 1)
            emit_merge(1)

    nc.compile()
    return nc


_OUTT = {}


def _shard_inputs(inputs):
    q = np.asarray(inputs["query"], np.float32)
    k = np.asarray(inputs["key"], np.float32)
    v = np.asarray(inputs["value"], np.float32)
    mask = np.asarray(inputs["mask"], np.int32)
    Wq = np.asarray(inputs["Wq"], np.float32)
    Wk = np.asarray(inputs["Wk"], np.float32)
    Wv = np.asarray(inputs["Wv"], np.float32)
    Wm = np.asarray(inputs["Wm"], np.float32)
    bq = np.asarray(inputs["bq"], np.float32)
    bk = np.asarray(inputs["bk"], np.float32)
    bv = np.asarray(inputs["bv"], np.float32)
    bm = np.asarray(inputs["bm"], np.float32)
    Aq = np.asarray(inputs["lora_A_q"], np.float32)
    Bq = np.asarray(inputs["lora_B_q"], np.float32)
    Av = np.asarray(inputs["lora_A_v"], np.float32)
    Bv = np.asarray(inputs["lora_B_v"], np.float32)

    import ml_dtypes
    bf16 = ml_dtypes.bfloat16

    def c(x):
        return np.ascontiguousarray(x)

    def cb(x):
        return np.ascontiguousarray(x.astype(bf16))

    def pack_sd(xT, nchunk):
        # [D, S'] -> [nchunk, 128, DO, S'/nchunk] with partition = d % 128
        sp = xT.shape[1] // nchunk
        return cb(xT.reshape(DO, 128, nchunk, sp).transpose(2, 1, 0, 3))

    def pack_w(wT, width):
        # [D, width] -> [128, DO, width]
        return cb(wT.reshape(DO, 128, width).transpose(1, 0, 2))

    qT = [np.ascontiguousarray(q[b].T) for b in range(B)]
    kT = [np.ascontiguousarray(k[b].T) for b in range(B)]
    vT = [np.ascontiguousarray(v[b].T) for b in range(B)]
    mT = [np.ascontiguousarray(mask[b].T) for b in range(B)]
    WqT, WkT, WvT, WmT = Wq.T, Wk.T, Wv.T, Wm.T
    BqT, BvT = Bq.T, Bv.T

    qt4 = [pack_sd(qT[b], NSC) for b in range(B)]
    kt4 = [pack_sd(kT[b], NSC) for b in range(B)]

    in_maps = []
    for core in range(NCORES):
        b, qh, hh = core // 4, (core // 2) % 2, core % 2
        hsl = slice(hh * HDIM, (hh + 1) * HDIM)
        qrows = slice(qh * QR, (qh + 1) * QR)
        # mask2: [S, QR] -> [NQC, 128, NKT, SC], partition = key % 128
        m = mT[b][:, qrows]
        mask2 = cb(m.reshape(NKT, 128, NQC, SC).transpose(2, 1, 0, 3))
        in_maps.append({
            "qt4": qt4[b],
            "kt4": kt4[b],
            "vth2": pack_sd(vT[b][:, qrows], NQC),
            "mask2": mask2,
            "wqt": pack_w(np.ascontiguousarray(WqT[:, hsl]), HDIM),
            "wkt": pack_w(np.ascontiguousarray(WkT[:, hsl]), HDIM),
            "wvt": pack_w(np.ascontiguousarray(WvT[:, hsl]), HDIM),
            "wmt": cb(WmT[hsl, :].reshape(3, 128, D).transpose(1, 0, 2)),
            "bqc": c(bq[hsl].reshape(3, 128).T),
            "bkc": c(bk[hsl].reshape(3, 128).T),
            "bvr": cb(bv[hsl].reshape(1, HDIM)),
            "bmr": cb((bm if hh == 0 else np.zeros_like(bm)).reshape(1, D)),
            "aq": pack_w(Aq, R), "av": pack_w(Av, R),
            "bqt": pack_w(BqT, R), "bvt": pack_w(BvT, R),
        })
    return in_maps


def _get_nc():
    if "nc" not in _CACHE:
        _CACHE["nc"] = _build_kernel()
    return _CACHE["nc"]


def kernel(**inputs) -> np.ndarray:
    nc = _get_nc()
    in_maps = _shard_inputs(inputs)
    res = run_bass_kernel_spmd(nc, in_maps, core_ids=list(range(NCORES)))
    out = np.zeros((B, S, D), np.float32)
    for b in range(B):
        for qh in range(2):
            part = (res.results[b * 4 + qh * 2 + 0]["out"]
                    + res.results[b * 4 + qh * 2 + 1]["out"])
            out[b, qh * QR:(qh + 1) * QR, :] = part
    return out


# revision 69
# speedup vs baseline: 1.0322x; 1.0322x over previous
"""Trainium2 Bass kernel for LoRA multi-head attention (B=2, S=2048, D=768, H=12, R=8).

Sharding over 8 cores: (batch, query-half, head-half) -> each core computes
6 heads x 1024 query rows x full 2048 keys, producing a partial (over the
head dimension) of the final merge projection. Host sums the two head-half
partials per (batch, query-half) slice.

All activations are kept feature-major ("transposed") on device so every
matmul contraction lands on the partition axis with no on-device transposes.
Host pre-packs every DRAM tensor so each DMA is one contiguous run per
partition (single descriptor per partition), split across the sync, scalar
and gpsimd queues.

The emission is software-pipelined around a continuous scalar-engine exp
stream (the steady-state bottleneck: 12.6M score elements per core):
half-pair scores (8 key tiles x 2 concurrently row-tiled heads) feed exp
directly into att tiles; mask multiplies are deferred, batched DVE ops;
v-projection and merge chunks ride a filler queue inside the scores halves
so the PE never idles while the scores PSUM ring is Act-paced. attv
matmuls append a shared all-ones block to v so each accumulation also
produces the softmax denominator replicated across 64 PSUM rows; the
normalize is then copy+reciprocal+one multiply on DVE. Dummy LDWEIGHTS
padding keeps the PE HAM clock-gate at full rate through Act-bound
stretches.
"""

import sys

if "/opt/trn_rl_repo" not in sys.path:
    sys.path.insert(0, "/opt/trn_rl_repo")

import numpy as np

import concourse.bass as bass
import concourse.tile as tile
from concourse import bacc, mybir
from concourse.bass_utils import run_bass_kernel_spmd

F32 = mybir.dt.float32
F32R = mybir.dt.float32r
BF16 = mybir.dt.bfloat16
I32 = mybir.dt.int32
EXP = mybir.ActivationFunctionType.Exp

B, S, D, H, R = 2, 2048, 768, 12, 8
DK = D // H  # 64
NCORES = 8
HPC = 6            # heads per core
HDIM = HPC * DK    # 384: head-slice width per core
QR = S // 2        # 1024 query rows per core
SC = 512           # streaming chunk (s dimension)
NSC = S // SC      # 4
NQC = QR // SC     # 2 query chunks per core
NKT = S // 128     # 16 key tiles
DO = D // 128      # 6 d-chunks

_CACHE = {}


def _build_kernel():
    """Build the full Bass program. One SPMD program serves all 8 cores; the
    (batch, q-half, head-half) selection is done host-side via input slicing.

    Emission order is software-pipelined so the scalar engine's exp chain
    starts right after the k/q projections, overlapping the v projection:
      pass1: k-proj, BW, uv, q-proj
      scores(q0,p0), scores(q1,p0)
      pass2: uq + v-proj
      attv/scores interleaved tail, merges
    """
    nc = bacc.Bacc("TRN2", target_bir_lowering=False, debug=False,
                   enable_asserts=True, num_devices=NCORES)

    def din(name, shape, dt=BF16):
        return nc.dram_tensor(name, shape, dt, kind="ExternalInput").ap()

    # all big inputs pre-packed host-side: leading chunk dim, then partition
    kt4_d = din("kt4", [NSC, 128, DO, SC])
    qt4_d = din("qt4", [NSC, 128, DO, SC])
    vth2_d = din("vth2", [NQC, 128, DO, SC])
    mask2_d = din("mask2", [NQC, 128, NKT, SC])
    wqt_d = din("wqt", [128, DO, HDIM])
    wkt_d = din("wkt", [128, DO, HDIM])
    wvt_d = din("wvt", [128, DO, HDIM])
    wmt_d = din("wmt", [128, 3, D])
    bqc_d = din("bqc", [128, 3], F32)
    bkc_d = din("bkc", [128, 3], F32)
    bvr_d = din("bvr", [1, HDIM])
    bmr_d = din("bmr", [1, D])
    aq_d = din("aq", [128, DO, R])
    av_d = din("av", [128, DO, R])
    bqt_d = din("bqt", [128, DO, R])
    bvt_d = din("bvt", [128, DO, R])
    out_d = nc.dram_tensor("out", [QR, D], F32, kind="ExternalOutput").ap()

    with tile.TileContext(nc) as tc:
        with (
            tc.tile_pool(name="keep", bufs=1) as keep,
            tc.tile_pool(name="maskp", bufs=2) as maskp,
            tc.tile_pool(name="wa", bufs=1) as wa,
            tc.tile_pool(name="acts", bufs=2) as acts,
            tc.tile_pool(name="kpool", bufs=2) as kpool,
            tc.tile_pool(name="bpool", bufs=2) as bpool,
            tc.tile_pool(name="apool", bufs=5) as apool,
            tc.tile_pool(name="npool", bufs=1) as npool,
            tc.tile_pool(name="fpool", bufs=2) as fpool,
            tc.tile_pool(name="psproj", bufs=2, space="PSUM") as psproj,
            tc.tile_pool(name="pss", bufs=2, space="PSUM") as pss,
            tc.tile_pool(name="pso", bufs=2, space="PSUM") as pso,
        ):
            # ---- persistent tiles ----
            ones_sb = keep.tile([1, 128], BF16)
            nc.vector.memset(ones_sb[:], 1.0)
            bm_sb = keep.tile([1, D], BF16)
            # kT split per s-chunk so scores can start on partial projections
            kT_sb = [keep.tile([128, 3, SC], BF16, name=f"kT{i}")
                     for i in range(NSC)]
            qT_sb = [keep.tile([128, 3, SC], BF16, name=f"qT{i}")
                     for i in range(NQC)]
            # v_aug per head pair: [v_even | ones | v_odd]; head 2p reads
            # cols 0:128 (num rows 0-63, den rows 64-127), head 2p+1 reads
            # cols 64:192 (den rows 0-63, num rows 64-127). The shared ones
            # block makes every attv matmul M=128 and yields the softmax
            # denominator replicated across 64 PSUM partitions for free.
            v_aug = keep.tile([128, NKT, HPC // 2, 3 * DK], BF16)
            nc.vector.memset(v_aug[:, :, :, DK:2 * DK], 1.0)

            # ---- prologue DMAs ----
            # sync queue: the k-projection critical path first; kTc2/3 go on
            # the scalar queue so their buffer-reuse waits don't block the
            # vThc loads behind them (head-of-line blocking).
            kTcs = []
            t = kpool.tile([128, DO, SC], BF16, tag="act", name="kTc0")
            nc.sync.dma_start(t[:], kt4_d[0])
            kTcs.append(t)
            wk_sb = wa.tile([128, DO, HDIM], BF16, name="wk_sb")
            nc.sync.dma_start(wk_sb[:], wkt_d[:])
            bk_sb = wa.tile([128, 3], F32)
            nc.sync.dma_start(bk_sb[:], bkc_d[:])
            bq_sb = wa.tile([128, 3], F32)
            nc.sync.dma_start(bq_sb[:], bqc_d[:])
            t = kpool.tile([128, DO, SC], BF16, tag="act", name="kTc1")
            nc.sync.dma_start(t[:], kt4_d[1])
            kTcs.append(t)
            vThcs = []
            for qc in range(NQC):
                t = acts.tile([128, DO, SC], BF16, tag="act2", name=f"vThc{qc}")
                nc.sync.dma_start(t[:], vth2_d[qc])
                vThcs.append(t)
            # kTc2/3 reuse ring buffers; scalar queue so their waits don't
            # block the sync queue
            for sc in (2, 3):
                t = kpool.tile([128, DO, SC], BF16, tag="act", name=f"kTc{sc}")
                nc.scalar.dma_start(t[:], kt4_d[sc])
                kTcs.append(t)

            # gpsimd queue: weights, lora, masks (otherwise idle engine)
            def gload(name, dram, shape, dt=BF16):
                t = wa.tile(shape, dt, name=name)
                nc.gpsimd.dma_start(t[:], dram[:])
                return t

            # ordered by when each transfer is first needed on-device
            wq_sb = gload("wq_sb", wqt_d, [128, DO, HDIM])
            bqt_sb = gload("bqt_sb", bqt_d, [128, DO, R])
            bvt_sb = gload("bvt_sb", bvt_d, [128, DO, R])
            av_sb = gload("av_sb", av_d, [128, DO, R])
            m01s = {}
            m01 = maskp.tile([128, NKT, SC], BF16, tag="mb", name="m01")
            nc.gpsimd.dma_start(m01[:], mask2_d[0])
            m01s[0] = m01
            bv_sb = gload("bv_sb", bvr_d, [1, HDIM])
            wv_sb = gload("wv_sb", wvt_d, [128, DO, HDIM])
            m01 = maskp.tile([128, NKT, SC], BF16, tag="mb", name="m01")
            nc.gpsimd.dma_start(m01[:], mask2_d[1])
            m01s[1] = m01
            aq_sb = gload("aq_sb", aq_d, [128, DO, R])
            nc.gpsimd.dma_start(bm_sb[:], bmr_d[:])

            BWq_sb = wa.tile([R, HDIM], BF16)
            BWv_sb = wa.tile([R, HDIM], BF16)

            # ---- pass 1 helpers ----
            def emit_kproj(sc):
                kTc = kTcs[sc]
                for et in range(3):
                    esl = slice(et * 128, (et + 1) * 128)
                    ps = psproj.tile([128, 512], F32, tag="pp", name="ps_k")
                    for do in range(DO):
                        nc.tensor.matmul(ps[:], wk_sb[:, do, esl], kTc[:, do, :],
                                         start=(do == 0), stop=(do == DO - 1))
                    nc.vector.tensor_scalar_add(
                        kT_sb[sc][:, et, :], ps[:], bk_sb[:, et:et + 1])

            def emit_bwq():
                ps = psproj.tile([128, 512], F32, tag="pp", name="ps_bwq")
                for do in range(DO):
                    nc.tensor.matmul(ps[:R, :HDIM], bvt_sb[:, do, :],
                                     wq_sb[:, do, :],
                                     start=(do == 0), stop=(do == DO - 1))
                nc.vector.tensor_copy(BWq_sb[:], ps[:R, :HDIM])

            def emit_bwv():
                ps = psproj.tile([128, 512], F32, tag="pp", name="ps_bwv")
                for do in range(DO):
                    nc.tensor.matmul(ps[:R, :HDIM], bqt_sb[:, do, :],
                                     wv_sb[:, do, :],
                                     start=(do == 0), stop=(do == DO - 1))
                nc.vector.tensor_copy(BWv_sb[:], ps[:R, :HDIM])

            def emit_qproj(qc):
                qsl = slice(qc * SC, (qc + 1) * SC)
                vThc = vThcs[qc]
                ps = psproj.tile([128, 512], F32, tag="pp", name="ps_uv")
                for do in range(DO):
                    nc.tensor.matmul(ps[:R, :], av_sb[:, do, :], vThc[:, do, :],
                                     start=(do == 0), stop=(do == DO - 1))
                uv_t = npool.tile([R, SC], BF16, tag="uv", name="uv_t")
                nc.vector.tensor_copy(uv_t[:], ps[:R, :])
                for et in range(3):
                    esl = slice(et * 128, (et + 1) * 128)
                    ps = psproj.tile([128, 512], F32, tag="pp", name="ps_q")
                    for do in range(DO):
                        nc.tensor.matmul(ps[:], wq_sb[:, do, esl], vThc[:, do, :],
                                         start=(do == 0), stop=False)
                    nc.tensor.matmul(ps[:], BWq_sb[:, esl], uv_t[:],
                                     start=False, stop=True)
                    nc.vector.tensor_scalar_add(
                        qT_sb[qc][:, et, :], ps[:], bq_sb[:, et:et + 1])

            # ---- phase B helpers ----
            atts = {}

            pso_tiles = {}
            HKT = NKT // 2  # 8 key tiles per half

            fillers = []

            def fill(n):
                for _ in range(min(n, len(fillers))):
                    fillers.pop(0)()

            def flush_fillers():
                while fillers:
                    fillers.pop(0)()

            def emit_scores_half(qc, p, hh, nfill=0, pad=False,
                                 mask_inline=False):
                # heads 2p (rows 0-63) and 2p+1 (rows 64-127) interleave as PE
                # row-groups and run concurrently; exp writes straight into
                # the att tile — the mask multiply is applied in-place later
                # (emit_mask_half) so it doesn't couple the DVE FIFO to the
                # exp stream. Filler units (pass2/merge chunks) are emitted
                # between kt groups so the PE stays busy while the scores
                # PSUM ring is paced by the Act engine.
                att = apool.tile([128, HKT, 2, SC], BF16, tag="att", name="att")
                atts[(qc, p, hh)] = att
                for i, kt in enumerate(range(hh * HKT, (hh + 1) * HKT)):
                    ps_s = pss.tile([128, 2, SC], F32, tag="ss", name="ps_s")
                    for j in range(2):
                        po = j * 64
                        nc.tensor.matmul(
                            ps_s[:, j, :],
                            kT_sb[kt // 4][po:po + 64, p,
                                           (kt % 4) * 128:(kt % 4 + 1) * 128],
                            qT_sb[qc][po:po + 64, p, :],
                            start=True, stop=True)
                    nc.scalar.activation(att[:, kt - hh * HKT, :, :], ps_s[:],
                                         EXP, scale=0.125)
                    if mask_inline:
                        ktl = kt - hh * HKT
                        mk = m01s[qc][:, kt, None, :].to_broadcast([128, 2, SC])
                        nc.vector.tensor_mul(att[:, ktl, :, :],
                                             att[:, ktl, :, :], mk)
                    if i < nfill:
                        fill(1)
                    elif pad:
                        for _ in range(4):
                            nc.tensor.ldweights(ones_sb[:])

            def emit_mask_half(qc, p, hh):
                m01 = m01s[qc]
                att = atts[(qc, p, hh)]
                for half in range(2):
                    a = half * (HKT // 2)
                    b = a + HKT // 2
                    mk = m01[:, hh * HKT + a:hh * HKT + b, None, :] \
                        .to_broadcast([128, HKT // 2, 2, SC])
                    nc.vector.tensor_mul(att[:, a:b, :, :], att[:, a:b, :, :],
                                         mk)

            def emit_attv_half(qc, p, hh):
                # attv accumulation spans both halves of the same PSUM tile;
                # other matmuls to other banks may interleave in between.
                att = atts.pop((qc, p, hh))
                for j in range(2):
                    if hh == 0:
                        pso_tiles[(qc, p, j)] = pso.tile([128, SC], F32,
                                                         tag="oo", name="ps_o")
                    ps_o = pso_tiles[(qc, p, j)]
                    for kt in range(hh * HKT, (hh + 1) * HKT):
                        nc.tensor.matmul(
                            ps_o[:],
                            v_aug[:, kt, p, j * DK:j * DK + 128],
                            att[:, kt - hh * HKT, j, :],
                            start=(kt == 0), stop=(kt == NKT - 1))
                    if hh == 1:
                        h = 2 * p + j
                        po = j * 64
                        # j=0: num rows 0-63, den rows 64-127; j=1 swapped.
                        no, do_ = (0, 64) if j == 0 else (64, 0)
                        ps_o = pso_tiles.pop((qc, p, j))
                        den_s = npool.tile([DK, SC], F32, tag="dn", name="den_s")
                        nc.vector.tensor_copy(den_s[:], ps_o[do_:do_ + DK, :])
                        rb = npool.tile([DK, SC], F32, tag="rb", name="rb")
                        nc.vector.reciprocal_approx_fast(rb[:], den_s[:])
                        for _ in range(4):
                            nc.tensor.ldweights(ones_sb[:])
                        if h == 0:
                            _OUTT[qc] = bpool.tile([128, 3, SC], BF16,
                                                   tag="outT", name="outT")
                        outT_sb = _OUTT[qc]
                        nc.vector.tensor_mul(outT_sb[po:po + 64, p, :],
                                             ps_o[no:no + DK, :], rb[:])

            def merge_chunk(qc, qt, ec):
                outT_sb = _OUTT[qc]
                qtsl = slice(qt * 128, (qt + 1) * 128)
                esl = slice(ec * 384, (ec + 1) * 384)
                ps_m = psproj.tile([128, 512], F32, tag="pp", name="ps_m")
                for hp in range(3):
                    nc.tensor.matmul(ps_m[:, :384], outT_sb[:, hp, qtsl],
                                     wm_sb[:, hp, esl],
                                     start=(hp == 0), stop=False)
                nc.tensor.matmul(ps_m[:, :384], ones_sb[:], bm_sb[:, esl],
                                 start=False, stop=True)
                fin = fpool.tile([128, 384], F32, tag="fin", name="fin")
                nc.vector.tensor_copy(fin[:], ps_m[:, :384])
                nc.sync.dma_start(
                    out_d[qc * SC + qt * 128:qc * SC + (qt + 1) * 128, esl],
                    fin[:])
                for _ in range(3):
                    nc.tensor.ldweights(ones_sb[:])

            def emit_merge(qc):
                for qt in range(4):
                    for ec in range(2):
                        merge_chunk(qc, qt, ec)

            def push_merge(qc):
                for qt in range(4):
                    for ec in range(2):
                        fillers.append(
                            lambda qt=qt, ec=ec: merge_chunk(qc, qt, ec))

            # ---- pass 2: uq + v-projection (natural layout), interleaved
            # with the first scores pairs so ACT/DVE fill early without the
            # projection copybacks queueing behind the mask multiplies ----
            qTcs = {}

            def emit_pass2_dma(sc):
                qTc = acts.tile([128, DO, SC], BF16, tag="act2", name="qTc")
                nc.sync.dma_start(qTc[:], qt4_d[sc])
                qTcs[sc] = qTc

            uqts = {}

            def p2_uq(sc):
                qTc = qTcs[sc]
                ps = psproj.tile([128, 512], F32, tag="pp", name="ps_uq")
                for do in range(DO):
                    nc.tensor.matmul(ps[:R, :], aq_sb[:, do, :], qTc[:, do, :],
                                     start=(do == 0), stop=(do == DO - 1))
                uq_t = npool.tile([R, SC], BF16, tag="uq", name="uq_t")
                nc.vector.tensor_copy(uq_t[:], ps[:R, :])
                uqts[sc] = uq_t

            def p2_st(sc, st):
                qTc = qTcs[sc]
                uq_t = uqts[sc]
                gst = sc * 4 + st
                stsl = slice(st * 128, (st + 1) * 128)
                ps = psproj.tile([128, 512], F32, tag="pp", name="ps_v")
                for do in range(DO):
                    nc.tensor.matmul(ps[:, :HDIM], qTc[:, do, stsl],
                                     wv_sb[:, do, :],
                                     start=(do == 0), stop=False)
                nc.tensor.matmul(ps[:, :HDIM],
                                 uq_t[:, st * 128:(st + 1) * 128], BWv_sb[:],
                                 start=False, stop=False)
                nc.tensor.matmul(ps[:, :HDIM], ones_sb[:], bv_sb[:],
                                 start=False, stop=True)
                sv = ps[:, :HDIM].rearrange("p (g two d) -> p g two d",
                                            g=3, two=2)
                nc.vector.tensor_copy(v_aug[:, gst, :, 0:DK], sv[:, :, 0, :])
                nc.vector.tensor_copy(v_aug[:, gst, :, 2 * DK:3 * DK],
                                      sv[:, :, 1, :])

            def push_p2(sc):
                fillers.append(lambda: p2_uq(sc))
                for st in range(4):
                    fillers.append(lambda st=st: p2_st(sc, st))

            # ---- schedule: projections first (DVE copybacks run while the
            # Act engine is still idle), then a continuous exp stream with
            # attv/pass2/merge work packed under it ----
            emit_kproj(0)
            emit_kproj(1)
            emit_bwq()
            emit_qproj(0)
            emit_scores_half(0, 0, 0)
            emit_kproj(2)
            emit_kproj(3)
            wm_sb = wk_sb[:].rearrange("p (u v) c -> p u (v c)", u=3, v=2)
            nc.gpsimd.dma_start(wm_sb, wmt_d[:])
            emit_qproj(1)
            emit_scores_half(0, 0, 1)
            emit_pass2_dma(0)
            emit_pass2_dma(1)
            emit_bwv()
            push_p2(0)
            push_p2(1)
            emit_scores_half(1, 0, 0, nfill=5)
            emit_pass2_dma(2)
            push_p2(2)
            emit_scores_half(1, 0, 1, nfill=6)
            emit_pass2_dma(3)
            push_p2(3)
            emit_mask_half(0, 0, 0)
            emit_attv_half(0, 0, 0)
            emit_scores_half(0, 1, 0, nfill=6)
            flush_fillers()
            emit_mask_half(0, 0, 1)
            emit_attv_half(0, 0, 1)
            emit_scores_half(0, 1, 1, pad=True)
            emit_mask_half(1, 0, 0)
            emit_attv_half(1, 0, 0)
            emit_scores_half(1, 1, 0, pad=True)
            emit_mask_half(1, 0, 1)
            emit_attv_half(1, 0, 1)
            emit_scores_half(1, 1, 1, pad=True)
            emit_mask_half(0, 1, 0)
            emit_attv_half(0, 1, 0)
            emit_scores_half(0, 2, 0, pad=True, mask_inline=True)
            emit_mask_half(0, 1, 1)
            emit_attv_half(0, 1, 1)
            emit_scores_half(0, 2, 1, pad=True, mask_inline=True)
            emit_mask_half(1, 1, 0)
            emit_attv_half(1, 1, 0)
            emit_scores_half(1, 2, 0, pad=True, mask_inline=True)
            emit_mask_half(1, 1, 1)
            emit_attv_half(1, 1, 1)
            emit_scores_half(1, 2, 1, pad=True, mask_inline=True)
            emit_attv_half(0, 2, 0)
            emit_attv_half(0, 2, 1)
            emit_merge(0)
            emit_attv_half(1, 2, 0)
            emit_attv_half(1, 2, 1)
            emit_merge(1)

    nc.compile()
    return nc


_OUTT = {}


def _shard_inputs(inputs):
    q = np.asarray(inputs["query"], np.float32)
    k = np.asarray(inputs["key"], np.float32)
    v = np.asarray(inputs["value"], np.float32)
    mask = np.asarray(inputs["mask"], np.int32)
    Wq = np.asarray(inputs["Wq"], np.float32)
    Wk = np.asarray(inputs["Wk"], np.float32)
    Wv = np.asarray(inputs["Wv"], np.float32)
    Wm = np.asarray(inputs["Wm"], np.float32)
    bq = np.asarray(inputs["bq"], np.float32)
    bk = np.asarray(inputs["bk"], np.float32)
    bv = np.asarray(inputs["bv"], np.float32)
    bm = np.asarray(inputs["bm"], np.float32)
    Aq = np.asarray(inputs["lora_A_q"], np.float32)
    Bq = np.asarray(inputs["lora_B_q"], np.float32)
    Av = np.asarray(inputs["lora_A_v"], np.float32)
    Bv = np.asarray(inputs["lora_B_v"], np.float32)

    import ml_dtypes
    bf16 = ml_dtypes.bfloat16

    def c(x):
        return np.ascontiguousarray(x)

    def cb(x):
        return np.ascontiguousarray(x.astype(bf16))

    def pack_sd(xT, nchunk):
        # [D, S'] -> [nchunk, 128, DO, S'/nchunk] with partition = d % 128
        sp = xT.shape[1] // nchunk
        return cb(xT.reshape(DO, 128, nchunk, sp).transpose(2, 1, 0, 3))

    def pack_w(wT, width):
        # [D, width] -> [128, DO, width]
        return cb(wT.reshape(DO, 128, width).transpose(1, 0, 2))

    qT = [np.ascontiguousarray(q[b].T) for b in range(B)]
    kT = [np.ascontiguousarray(k[b].T) for b in range(B)]
    vT = [np.ascontiguousarray(v[b].T) for b in range(B)]
    mT = [np.ascontiguousarray(mask[b].T) for b in range(B)]
    WqT, WkT, WvT, WmT = Wq.T, Wk.T, Wv.T, Wm.T
    BqT, BvT = Bq.T, Bv.T

    qt4 = [pack_sd(qT[b], NSC) for b in range(B)]
    kt4 = [pack_sd(kT[b], NSC) for b in range(B)]

    in_maps = []
    for core in range(NCORES):
        b, qh, hh = core // 4, (core // 2) % 2, core % 2
        hsl = slice(hh * HDIM, (hh + 1) * HDIM)
        qrows = slice(qh * QR, (qh + 1) * QR)
        # mask2: [S, QR] -> [NQC, 128, NKT, SC], partition = key % 128
        m = mT[b][:, qrows]
        mask2 = cb(m.reshape(NKT, 128, NQC, SC).transpose(2, 1, 0, 3))
        in_maps.append({
            "qt4": qt4[b],
            "kt4": kt4[b],
            "vth2": pack_sd(vT[b][:, qrows], NQC),
            "mask2": mask2,
            "wqt": pack_w(np.ascontiguousarray(WqT[:, hsl]), HDIM),
            "wkt": pack_w(np.ascontiguousarray(WkT[:, hsl]), HDIM),
            "wvt": pack_w(np.ascontiguousarray(WvT[:, hsl]), HDIM),
            "wmt": cb(WmT[hsl, :].reshape(3, 128, D).transpose(1, 0, 2)),
            "bqc": c(bq[hsl].reshape(3, 128).T),
            "bkc": c(bk[hsl].reshape(3, 128).T),
            "bvr": cb(bv[hsl].reshape(1, HDIM)),
            "bmr": cb((bm if hh == 0 else np.zeros_like(bm)).reshape(1, D)),
            "aq": pack_w(Aq, R), "av": pack_w(Av, R),
            "bqt": pack_w(BqT, R), "bvt": pack_w(BvT, R),
        })
    return in_maps


def _get_nc():
    if "nc" not in _CACHE:
        _CACHE["nc"] = _build_kernel()
    return _CACHE["nc"]


def kernel(**inputs) -> np.ndarray:
    nc = _get_nc()
    in_maps = _shard_inputs(inputs)
    res = run_bass_kernel_spmd(nc, in_maps, core_ids=list(range(NCORES)))
    out = np.zeros((B, S, D), np.float32)
    for b in range(B):
        for qh in range(2):
            part = (res.results[b * 4 + qh * 2 + 0]["out"]
                    + res.results[b * 4 + qh * 2 + 1]["out"])
            out[b, qh * QR:(qh + 1) * QR, :] = part
    return out


# revision 70
# speedup vs baseline: 1.0392x; 1.0068x over previous
"""Trainium2 Bass kernel for LoRA multi-head attention (B=2, S=2048, D=768, H=12, R=8).

Sharding over 8 cores: (batch, query-half, head-half) -> each core computes
6 heads x 1024 query rows x full 2048 keys, producing a partial (over the
head dimension) of the final merge projection. Host sums the two head-half
partials per (batch, query-half) slice.

All activations are kept feature-major ("transposed") on device so every
matmul contraction lands on the partition axis with no on-device transposes.
Host pre-packs every DRAM tensor so each DMA is one contiguous run per
partition (single descriptor per partition), split across the sync, scalar
and gpsimd queues.

The emission is software-pipelined around a continuous scalar-engine exp
stream (the steady-state bottleneck: 12.6M score elements per core):
half-pair scores (8 key tiles x 2 concurrently row-tiled heads) feed exp
directly into att tiles; mask multiplies are deferred, batched DVE ops;
v-projection and merge chunks ride a filler queue inside the scores halves
so the PE never idles while the scores PSUM ring is Act-paced. attv
matmuls append a shared all-ones block to v so each accumulation also
produces the softmax denominator replicated across 64 PSUM rows; the
normalize is then copy+reciprocal+one multiply on DVE. Dummy LDWEIGHTS
padding keeps the PE HAM clock-gate at full rate through Act-bound
stretches.
"""

import sys

if "/opt/trn_rl_repo" not in sys.path:
    sys.path.insert(0, "/opt/trn_rl_repo")

import numpy as np

import concourse.bass as bass
import concourse.tile as tile
from concourse import bacc, mybir
from concourse.bass_utils import run_bass_kernel_spmd

F32 = mybir.dt.float32
F32R = mybir.dt.float32r
BF16 = mybir.dt.bfloat16
I32 = mybir.dt.int32
EXP = mybir.ActivationFunctionType.Exp

B, S, D, H, R = 2, 2048, 768, 12, 8
DK = D // H  # 64
NCORES = 8
HPC = 6            # heads per core
HDIM = HPC * DK    # 384: head-slice width per core
QR = S // 2        # 1024 query rows per core
SC = 512           # streaming chunk (s dimension)
NSC = S // SC      # 4
NQC = QR // SC     # 2 query chunks per core
NKT = S // 128     # 16 key tiles
DO = D // 128      # 6 d-chunks

_CACHE = {}


def _build_kernel():
    """Build the full Bass program. One SPMD program serves all 8 cores; the
    (batch, q-half, head-half) selection is done host-side via input slicing.

    Emission order is software-pipelined so the scalar engine's exp chain
    starts right after the k/q projections, overlapping the v projection:
      pass1: k-proj, BW, uv, q-proj
      scores(q0,p0), scores(q1,p0)
      pass2: uq + v-proj
      attv/scores interleaved tail, merges
    """
    nc = bacc.Bacc("TRN2", target_bir_lowering=False, debug=False,
                   enable_asserts=True, num_devices=NCORES)

    def din(name, shape, dt=BF16):
        return nc.dram_tensor(name, shape, dt, kind="ExternalInput").ap()

    # all big inputs pre-packed host-side: leading chunk dim, then partition
    kt4_d = din("kt4", [NSC, 128, DO, SC])
    qt4_d = din("qt4", [NSC, 128, DO, SC])
    vth2_d = din("vth2", [NQC, 128, DO, SC])
    mask2_d = din("mask2", [NQC, 128, NKT, SC])
    wqt_d = din("wqt", [128, DO, HDIM])
    wkt_d = din("wkt", [128, DO, HDIM])
    wvt_d = din("wvt", [128, DO, HDIM])
    wmt_d = din("wmt", [128, 3, D])
    bqc_d = din("bqc", [128, 3], F32)
    bkc_d = din("bkc", [128, 3], F32)
    bvr_d = din("bvr", [1, HDIM])
    bmr_d = din("bmr", [1, D])
    aq_d = din("aq", [128, DO, R])
    av_d = din("av", [128, DO, R])
    bqt_d = din("bqt", [128, DO, R])
    bvt_d = din("bvt", [128, DO, R])
    out_d = nc.dram_tensor("out", [QR, D], F32, kind="ExternalOutput").ap()

    with tile.TileContext(nc) as tc:
        with (
            tc.tile_pool(name="keep", bufs=1) as keep,
            tc.tile_pool(name="maskp", bufs=2) as maskp,
            tc.tile_pool(name="wa", bufs=1) as wa,
            tc.tile_pool(name="acts", bufs=2) as acts,
            tc.tile_pool(name="kpool", bufs=2) as kpool,
            tc.tile_pool(name="bpool", bufs=2) as bpool,
            tc.tile_pool(name="apool", bufs=5) as apool,
            tc.tile_pool(name="npool", bufs=1) as npool,
            tc.tile_pool(name="fpool", bufs=2) as fpool,
            tc.tile_pool(name="psproj", bufs=2, space="PSUM") as psproj,
            tc.tile_pool(name="pss", bufs=2, space="PSUM") as pss,
            tc.tile_pool(name="pso", bufs=2, space="PSUM") as pso,
        ):
            # ---- persistent tiles ----
            ones_sb = keep.tile([1, 128], BF16)
            nc.vector.memset(ones_sb[:], 1.0)
            bm_sb = keep.tile([1, D], BF16)
            # kT split per s-chunk so scores can start on partial projections
            kT_sb = [keep.tile([128, 3, SC], BF16, name=f"kT{i}")
                     for i in range(NSC)]
            qT_sb = [keep.tile([128, 3, SC], BF16, name=f"qT{i}")
                     for i in range(NQC)]
            # v_aug per head pair: [v_even | ones | v_odd]; head 2p reads
            # cols 0:128 (num rows 0-63, den rows 64-127), head 2p+1 reads
            # cols 64:192 (den rows 0-63, num rows 64-127). The shared ones
            # block makes every attv matmul M=128 and yields the softmax
            # denominator replicated across 64 PSUM partitions for free.
            v_aug = keep.tile([128, NKT, HPC // 2, 3 * DK], BF16)
            nc.vector.memset(v_aug[:, :, :, DK:2 * DK], 1.0)

            # ---- prologue DMAs ----
            # sync queue: the k-projection critical path first; kTc2/3 go on
            # the scalar queue so their buffer-reuse waits don't block the
            # vThc loads behind them (head-of-line blocking).
            kTcs = []
            t = kpool.tile([128, DO, SC], BF16, tag="act", name="kTc0")
            nc.sync.dma_start(t[:], kt4_d[0])
            kTcs.append(t)
            wk_sb = wa.tile([128, DO, HDIM], BF16, name="wk_sb")
            nc.sync.dma_start(wk_sb[:], wkt_d[:])
            bk_sb = wa.tile([128, 3], F32)
            nc.sync.dma_start(bk_sb[:], bkc_d[:])
            bq_sb = wa.tile([128, 3], F32)
            nc.sync.dma_start(bq_sb[:], bqc_d[:])
            t = kpool.tile([128, DO, SC], BF16, tag="act", name="kTc1")
            nc.sync.dma_start(t[:], kt4_d[1])
            kTcs.append(t)
            vThcs = []
            for qc in range(NQC):
                t = acts.tile([128, DO, SC], BF16, tag="act2", name=f"vThc{qc}")
                nc.sync.dma_start(t[:], vth2_d[qc])
                vThcs.append(t)
            # kTc2/3 reuse ring buffers; scalar queue so their waits don't
            # block the sync queue
            for sc in (2, 3):
                t = kpool.tile([128, DO, SC], BF16, tag="act", name=f"kTc{sc}")
                nc.scalar.dma_start(t[:], kt4_d[sc])
                kTcs.append(t)

            # gpsimd queue: weights, lora, masks (otherwise idle engine)
            def gload(name, dram, shape, dt=BF16):
                t = wa.tile(shape, dt, name=name)
                nc.gpsimd.dma_start(t[:], dram[:])
                return t

            # ordered by when each transfer is first needed on-device
            wq_sb = gload("wq_sb", wqt_d, [128, DO, HDIM])
            bqt_sb = gload("bqt_sb", bqt_d, [128, DO, R])
            bvt_sb = gload("bvt_sb", bvt_d, [128, DO, R])
            av_sb = gload("av_sb", av_d, [128, DO, R])
            m01s = {}
            m01 = maskp.tile([128, NKT, SC], BF16, tag="mb", name="m01")
            nc.gpsimd.dma_start(m01[:], mask2_d[0])
            m01s[0] = m01
            bv_sb = gload("bv_sb", bvr_d, [1, HDIM])
            wv_sb = gload("wv_sb", wvt_d, [128, DO, HDIM])
            m01 = maskp.tile([128, NKT, SC], BF16, tag="mb", name="m01")
            nc.gpsimd.dma_start(m01[:], mask2_d[1])
            m01s[1] = m01
            aq_sb = gload("aq_sb", aq_d, [128, DO, R])
            nc.gpsimd.dma_start(bm_sb[:], bmr_d[:])

            BWq_sb = wa.tile([R, HDIM], BF16)
            BWv_sb = wa.tile([R, HDIM], BF16)

            # ---- pass 1 helpers ----
            def emit_kproj(sc):
                kTc = kTcs[sc]
                for et in range(3):
                    esl = slice(et * 128, (et + 1) * 128)
                    ps = psproj.tile([128, 512], F32, tag="pp", name="ps_k")
                    for do in range(DO):
                        nc.tensor.matmul(ps[:], wk_sb[:, do, esl], kTc[:, do, :],
                                         start=(do == 0), stop=(do == DO - 1))
                    nc.vector.tensor_scalar_add(
                        kT_sb[sc][:, et, :], ps[:], bk_sb[:, et:et + 1])

            def emit_bwq():
                ps = psproj.tile([128, 512], F32, tag="pp", name="ps_bwq")
                for do in range(DO):
                    nc.tensor.matmul(ps[:R, :HDIM], bvt_sb[:, do, :],
                                     wq_sb[:, do, :],
                                     start=(do == 0), stop=(do == DO - 1))
                nc.vector.tensor_copy(BWq_sb[:], ps[:R, :HDIM])

            def emit_bwv():
                ps = psproj.tile([128, 512], F32, tag="pp", name="ps_bwv")
                for do in range(DO):
                    nc.tensor.matmul(ps[:R, :HDIM], bqt_sb[:, do, :],
                                     wv_sb[:, do, :],
                                     start=(do == 0), stop=(do == DO - 1))
                nc.vector.tensor_copy(BWv_sb[:], ps[:R, :HDIM])

            def emit_qproj(qc):
                qsl = slice(qc * SC, (qc + 1) * SC)
                vThc = vThcs[qc]
                ps = psproj.tile([128, 512], F32, tag="pp", name="ps_uv")
                for do in range(DO):
                    nc.tensor.matmul(ps[:R, :], av_sb[:, do, :], vThc[:, do, :],
                                     start=(do == 0), stop=(do == DO - 1))
                uv_t = npool.tile([R, SC], BF16, tag="uv", name="uv_t")
                nc.vector.tensor_copy(uv_t[:], ps[:R, :])
                for et in range(3):
                    esl = slice(et * 128, (et + 1) * 128)
                    ps = psproj.tile([128, 512], F32, tag="pp", name="ps_q")
                    for do in range(DO):
                        nc.tensor.matmul(ps[:], wq_sb[:, do, esl], vThc[:, do, :],
                                         start=(do == 0), stop=False)
                    nc.tensor.matmul(ps[:], BWq_sb[:, esl], uv_t[:],
                                     start=False, stop=True)
                    nc.vector.tensor_scalar_add(
                        qT_sb[qc][:, et, :], ps[:], bq_sb[:, et:et + 1])

            # ---- phase B helpers ----
            atts = {}

            pso_tiles = {}
            HKT = NKT // 2  # 8 key tiles per half

            fillers = []

            def fill(n):
                for _ in range(min(n, len(fillers))):
                    fillers.pop(0)()

            def flush_fillers():
                while fillers:
                    fillers.pop(0)()

            def emit_scores_half(qc, p, hh, nfill=0, pad=False,
                                 mask_inline=False):
                # heads 2p (rows 0-63) and 2p+1 (rows 64-127) interleave as PE
                # row-groups and run concurrently; exp writes straight into
                # the att tile — the mask multiply is applied in-place later
                # (emit_mask_half) so it doesn't couple the DVE FIFO to the
                # exp stream. Filler units (pass2/merge chunks) are emitted
                # between kt groups so the PE stays busy while the scores
                # PSUM ring is paced by the Act engine.
                att = apool.tile([128, HKT, 2, SC], BF16, tag="att", name="att")
                atts[(qc, p, hh)] = att
                for i, kt in enumerate(range(hh * HKT, (hh + 1) * HKT)):
                    ps_s = pss.tile([128, 2, SC], F32, tag="ss", name="ps_s")
                    for j in range(2):
                        po = j * 64
                        nc.tensor.matmul(
                            ps_s[:, j, :],
                            kT_sb[kt // 4][po:po + 64, p,
                                           (kt % 4) * 128:(kt % 4 + 1) * 128],
                            qT_sb[qc][po:po + 64, p, :],
                            start=True, stop=True)
                    nc.scalar.activation(att[:, kt - hh * HKT, :, :], ps_s[:],
                                         EXP, scale=0.125)
                    if mask_inline:
                        ktl = kt - hh * HKT
                        mk = m01s[qc][:, kt, None, :].to_broadcast([128, 2, SC])
                        nc.vector.tensor_mul(att[:, ktl, :, :],
                                             att[:, ktl, :, :], mk)
                    if i < nfill:
                        fill(1)
                    elif pad:
                        for _ in range(4):
                            nc.tensor.ldweights(ones_sb[:])

            def emit_mask_half(qc, p, hh):
                m01 = m01s[qc]
                att = atts[(qc, p, hh)]
                for half in range(2):
                    a = half * (HKT // 2)
                    b = a + HKT // 2
                    mk = m01[:, hh * HKT + a:hh * HKT + b, None, :] \
                        .to_broadcast([128, HKT // 2, 2, SC])
                    nc.vector.tensor_mul(att[:, a:b, :, :], att[:, a:b, :, :],
                                         mk)

            def emit_attv_half(qc, p, hh):
                # attv accumulation spans both halves of the same PSUM tile;
                # other matmuls to other banks may interleave in between.
                att = atts.pop((qc, p, hh))
                for j in range(2):
                    if hh == 0:
                        pso_tiles[(qc, p, j)] = pso.tile([128, SC], F32,
                                                         tag="oo", name="ps_o")
                    ps_o = pso_tiles[(qc, p, j)]
                    for kt in range(hh * HKT, (hh + 1) * HKT):
                        nc.tensor.matmul(
                            ps_o[:],
                            v_aug[:, kt, p, j * DK:j * DK + 128],
                            att[:, kt - hh * HKT, j, :],
                            start=(kt == 0), stop=(kt == NKT - 1))
                    if hh == 1:
                        h = 2 * p + j
                        po = j * 64
                        # j=0: num rows 0-63, den rows 64-127; j=1 swapped.
                        no, do_ = (0, 64) if j == 0 else (64, 0)
                        ps_o = pso_tiles.pop((qc, p, j))
                        den_s = npool.tile([DK, SC], F32, tag="dn", name="den_s")
                        nc.vector.tensor_copy(den_s[:], ps_o[do_:do_ + DK, :])
                        rb = npool.tile([DK, SC], F32, tag="rb", name="rb")
                        nc.vector.reciprocal_approx_fast(rb[:], den_s[:])
                        for _ in range(4):
                            nc.tensor.ldweights(ones_sb[:])
                        if h == 0:
                            _OUTT[qc] = bpool.tile([128, 3, SC], BF16,
                                                   tag="outT", name="outT")
                        outT_sb = _OUTT[qc]
                        nc.vector.tensor_mul(outT_sb[po:po + 64, p, :],
                                             ps_o[no:no + DK, :], rb[:])

            def merge_chunk(qc, qt, ec):
                outT_sb = _OUTT[qc]
                qtsl = slice(qt * 128, (qt + 1) * 128)
                esl = slice(ec * 384, (ec + 1) * 384)
                ps_m = psproj.tile([128, 512], F32, tag="pp", name="ps_m")
                for hp in range(3):
                    nc.tensor.matmul(ps_m[:, :384], outT_sb[:, hp, qtsl],
                                     wm_sb[:, hp, esl],
                                     start=(hp == 0), stop=False)
                nc.tensor.matmul(ps_m[:, :384], ones_sb[:], bm_sb[:, esl],
                                 start=False, stop=True)
                fin = fpool.tile([128, 384], F32, tag="fin", name="fin")
                nc.vector.tensor_copy(fin[:], ps_m[:, :384])
                nc.sync.dma_start(
                    out_d[qc * SC + qt * 128:qc * SC + (qt + 1) * 128, esl],
                    fin[:])
                for _ in range(3):
                    nc.tensor.ldweights(ones_sb[:])

            def emit_merge(qc):
                for qt in range(4):
                    for ec in range(2):
                        merge_chunk(qc, qt, ec)

            def push_merge(qc):
                for qt in range(4):
                    for ec in range(2):
                        fillers.append(
                            lambda qt=qt, ec=ec: merge_chunk(qc, qt, ec))

            # ---- pass 2: uq + v-projection (natural layout), interleaved
            # with the first scores pairs so ACT/DVE fill early without the
            # projection copybacks queueing behind the mask multiplies ----
            qTcs = {}

            def emit_pass2_dma(sc):
                qTc = acts.tile([128, DO, SC], BF16, tag="act2", name="qTc")
                nc.sync.dma_start(qTc[:], qt4_d[sc])
                qTcs[sc] = qTc

            uqts = {}

            def p2_uq(sc):
                qTc = qTcs[sc]
                ps = psproj.tile([128, 512], F32, tag="pp", name="ps_uq")
                for do in range(DO):
                    nc.tensor.matmul(ps[:R, :], aq_sb[:, do, :], qTc[:, do, :],
                                     start=(do == 0), stop=(do == DO - 1))
                uq_t = npool.tile([R, SC], BF16, tag="uq", name="uq_t")
                nc.vector.tensor_copy(uq_t[:], ps[:R, :])
                uqts[sc] = uq_t

            def p2_st(sc, st):
                qTc = qTcs[sc]
                uq_t = uqts[sc]
                gst = sc * 4 + st
                stsl = slice(st * 128, (st + 1) * 128)
                ps = psproj.tile([128, 512], F32, tag="pp", name="ps_v")
                for do in range(DO):
                    nc.tensor.matmul(ps[:, :HDIM], qTc[:, do, stsl],
                                     wv_sb[:, do, :],
                                     start=(do == 0), stop=False)
                nc.tensor.matmul(ps[:, :HDIM],
                                 uq_t[:, st * 128:(st + 1) * 128], BWv_sb[:],
                                 start=False, stop=False)
                nc.tensor.matmul(ps[:, :HDIM], ones_sb[:], bv_sb[:],
                                 start=False, stop=True)
                sv = ps[:, :HDIM].rearrange("p (g two d) -> p g two d",
                                            g=3, two=2)
                nc.vector.tensor_copy(v_aug[:, gst, :, 0:DK], sv[:, :, 0, :])
                nc.vector.tensor_copy(v_aug[:, gst, :, 2 * DK:3 * DK],
                                      sv[:, :, 1, :])

            def push_p2(sc):
                fillers.append(lambda: p2_uq(sc))
                for st in range(4):
                    fillers.append(lambda st=st: p2_st(sc, st))

            # ---- schedule: projections first (DVE copybacks run while the
            # Act engine is still idle), then a continuous exp stream with
            # attv/pass2/merge work packed under it ----
            emit_kproj(0)
            emit_kproj(1)
            emit_bwq()
            emit_qproj(0)
            emit_scores_half(0, 0, 0)
            emit_kproj(2)
            emit_kproj(3)
            wm_sb = wk_sb[:].rearrange("p (u v) c -> p u (v c)", u=3, v=2)
            nc.gpsimd.dma_start(wm_sb, wmt_d[:])
            emit_qproj(1)
            emit_scores_half(0, 0, 1)
            emit_pass2_dma(0)
            emit_pass2_dma(1)
            emit_bwv()
            push_p2(0)
            push_p2(1)
            emit_scores_half(1, 0, 0, nfill=5)
            emit_pass2_dma(2)
            push_p2(2)
            emit_scores_half(1, 0, 1, nfill=6)
            emit_pass2_dma(3)
            push_p2(3)
            emit_mask_half(0, 0, 0)
            emit_attv_half(0, 0, 0)
            emit_scores_half(0, 1, 0, nfill=8)
            flush_fillers()
            emit_mask_half(0, 0, 1)
            emit_attv_half(0, 0, 1)
            emit_scores_half(0, 1, 1, pad=True)
            emit_mask_half(1, 0, 0)
            emit_attv_half(1, 0, 0)
            emit_scores_half(1, 1, 0, pad=True)
            emit_mask_half(1, 0, 1)
            emit_attv_half(1, 0, 1)
            emit_scores_half(1, 1, 1, pad=True)
            emit_mask_half(0, 1, 0)
            emit_attv_half(0, 1, 0)
            emit_scores_half(0, 2, 0, pad=True, mask_inline=True)
            emit_mask_half(0, 1, 1)
            emit_attv_half(0, 1, 1)
            emit_scores_half(0, 2, 1, pad=True, mask_inline=True)
            emit_mask_half(1, 1, 0)
            emit_attv_half(1, 1, 0)
            emit_scores_half(1, 2, 0, pad=True, mask_inline=True)
            emit_mask_half(1, 1, 1)
            emit_attv_half(1, 1, 1)
            emit_scores_half(1, 2, 1, pad=True, mask_inline=True)
            emit_attv_half(0, 2, 0)
            emit_attv_half(0, 2, 1)
            emit_merge(0)
            emit_attv_half(1, 2, 0)
            emit_attv_half(1, 2, 1)
            emit_merge(1)

    nc.compile()
    return nc


_OUTT = {}


def _shard_inputs(inputs):
    q = np.asarray(inputs["query"], np.float32)
    k = np.asarray(inputs["key"], np.float32)
    v = np.asarray(inputs["value"], np.float32)
    mask = np.asarray(inputs["mask"], np.int32)
    Wq = np.asarray(inputs["Wq"], np.float32)
    Wk = np.asarray(inputs["Wk"], np.float32)
    Wv = np.asarray(inputs["Wv"], np.float32)
    Wm = np.asarray(inputs["Wm"], np.float32)
    bq = np.asarray(inputs["bq"], np.float32)
    bk = np.asarray(inputs["bk"], np.float32)
    bv = np.asarray(inputs["bv"], np.float32)
    bm = np.asarray(inputs["bm"], np.float32)
    Aq = np.asarray(inputs["lora_A_q"], np.float32)
    Bq = np.asarray(inputs["lora_B_q"], np.float32)
    Av = np.asarray(inputs["lora_A_v"], np.float32)
    Bv = np.asarray(inputs["lora_B_v"], np.float32)

    import ml_dtypes
    bf16 = ml_dtypes.bfloat16

    def c(x):
        return np.ascontiguousarray(x)

    def cb(x):
        return np.ascontiguousarray(x.astype(bf16))

    def pack_sd(xT, nchunk):
        # [D, S'] -> [nchunk, 128, DO, S'/nchunk] with partition = d % 128
        sp = xT.shape[1] // nchunk
        return cb(xT.reshape(DO, 128, nchunk, sp).transpose(2, 1, 0, 3))

    def pack_w(wT, width):
        # [D, width] -> [128, DO, width]
        return cb(wT.reshape(DO, 128, width).transpose(1, 0, 2))

    qT = [np.ascontiguousarray(q[b].T) for b in range(B)]
    kT = [np.ascontiguousarray(k[b].T) for b in range(B)]
    vT = [np.ascontiguousarray(v[b].T) for b in range(B)]
    mT = [np.ascontiguousarray(mask[b].T) for b in range(B)]
    WqT, WkT, WvT, WmT = Wq.T, Wk.T, Wv.T, Wm.T
    BqT, BvT = Bq.T, Bv.T

    qt4 = [pack_sd(qT[b], NSC) for b in range(B)]
    kt4 = [pack_sd(kT[b], NSC) for b in range(B)]

    in_maps = []
    for core in range(NCORES):
        b, qh, hh = core // 4, (core // 2) % 2, core % 2
        hsl = slice(hh * HDIM, (hh + 1) * HDIM)
        qrows = slice(qh * QR, (qh + 1) * QR)
        # mask2: [S, QR] -> [NQC, 128, NKT, SC], partition = key % 128
        m = mT[b][:, qrows]
        mask2 = cb(m.reshape(NKT, 128, NQC, SC).transpose(2, 1, 0, 3))
        in_maps.append({
            "qt4": qt4[b],
            "kt4": kt4[b],
            "vth2": pack_sd(vT[b][:, qrows], NQC),
            "mask2": mask2,
            "wqt": pack_w(np.ascontiguousarray(WqT[:, hsl]), HDIM),
            "wkt": pack_w(np.ascontiguousarray(WkT[:, hsl]), HDIM),
            "wvt": pack_w(np.ascontiguousarray(WvT[:, hsl]), HDIM),
            "wmt": cb(WmT[hsl, :].reshape(3, 128, D).transpose(1, 0, 2)),
            "bqc": c(bq[hsl].reshape(3, 128).T),
            "bkc": c(bk[hsl].reshape(3, 128).T),
            "bvr": cb(bv[hsl].reshape(1, HDIM)),
            "bmr": cb((bm if hh == 0 else np.zeros_like(bm)).reshape(1, D)),
            "aq": pack_w(Aq, R), "av": pack_w(Av, R),
            "bqt": pack_w(BqT, R), "bvt": pack_w(BvT, R),
        })
    return in_maps


def _get_nc():
    if "nc" not in _CACHE:
        _CACHE["nc"] = _build_kernel()
    return _CACHE["nc"]


def kernel(**inputs) -> np.ndarray:
    nc = _get_nc()
    in_maps = _shard_inputs(inputs)
    res = run_bass_kernel_spmd(nc, in_maps, core_ids=list(range(NCORES)))
    out = np.zeros((B, S, D), np.float32)
    for b in range(B):
        for qh in range(2):
            part = (res.results[b * 4 + qh * 2 + 0]["out"]
                    + res.results[b * 4 + qh * 2 + 1]["out"])
            out[b, qh * QR:(qh + 1) * QR, :] = part
    return out
